# revision 2
# baseline (speedup 1.0000x reference)
"""Trainium2 Bass kernel for nn_AttentionNeuralOperator (dense_transformer), v2.

Strategy (8 NeuronCores, data-parallel over the b*s=64 sequences, 8 per core):
  All matmuls in 16-bit (fp16 front-end / bf16 post-exp) with fp32 PSUM:
  fp16 has tf32-class mantissa so scores keep precision; E=exp(s)*exp(b) is
  bf16 for range. LDWEIGHTS runs 4x faster than fp32 (FWL), DMA bytes halve.
  Per sequence (c=128, L=576, 2 heads, d_qk=64, d_v=128):
    qk  = qk_wT.T @ x            -> q,k (d, L) fp16        [PE] -> DVE copies
    vT  = x.T @ v_wT             -> v (m, d) fp16          [PE]
    scoresT = k_h.T q_h          -> (m-part, l-free)       [PE, heads on
                                    disjoint row-groups, concurrent]
    expT = exp(scoresT) -> bf16  [ACT], multiplied by host-precomputed
      exp(bias) table (bf16), split DVE/GpSimd for engine balance.
    sums via ones-matmul, reciprocal via 32x32 transpose trick (bf16),
      broadcast via DRAM bounce.
    out_h = vT_h.T @ expT_h, normalized by 1/sums           [PE + DVE]
    1x1-conv MLP (fp16 weights) with exact GELU             [PE + ACT]
  y is fp16 on device; host upcasts to fp32.
"""
import sys
sys.path.insert(0, "/opt/trn_rl_repo")
import numpy as np
import ml_dtypes

import concourse.bass as bass
import concourse.tile as tile
from concourse.tile import add_dep_helper
from concourse import bacc, mybir
from concourse.bass_utils import run_bass_kernel_spmd

P = 128
HEADS = 2
B, S, C, HH, WW = 2, 32, 128, 24, 24
L = HH * WW            # 576
LP = 640               # m padded to 5*128
NSEQ = B * S           # 64
NCORES = 8
SEQ_PER_CORE = NSEQ // NCORES  # 8
HID = 256
OUT_CH = 128
QKD = C // HEADS       # 64
VD = HID // HEADS      # 128
NCH = LP // P          # 5 m-chunks
PIECE = 288            # l-piece
F32 = mybir.dt.float32
F16 = mybir.dt.float16
BF16 = mybir.dt.bfloat16
NPBF16 = ml_dtypes.bfloat16


def _log_cpb_np(h, w, w1, b1, w2):
    """Host fp32 mirror of the reference CPB MLP, on the 47x47 delta grid."""
    dy = np.arange(-(h - 1), h, dtype=np.float32)
    dx = np.arange(-(w - 1), w, dtype=np.float32)
    rel = np.stack(np.meshgrid(dy, dx, indexing="ij"), axis=-1)     # (2h-1, 2w-1, 2)
    denom = np.array([max(h - 1, 1), max(w - 1, 1)], dtype=np.float32)
    rel = rel / denom * np.float32(8.0)
    rel = np.sign(rel) * np.log2(np.float32(1.0) + np.abs(rel)) / np.float32(np.log2(8.0))
    hid_act = np.maximum(rel @ w1.T + b1, np.float32(0.0))          # (2h-1, 2w-1, c)
    tab = (hid_act @ w2.T).astype(np.float32)                       # (2h-1, 2w-1, heads)
    yl = np.repeat(np.arange(h), w)
    xl = np.tile(np.arange(w), h)
    DY = yl[:, None] - yl[None, :] + (h - 1)                        # (L, L)
    DX = xl[:, None] - xl[None, :] + (w - 1)
    return tab[DY, DX].transpose(2, 0, 1)                           # (heads, L, L)


def _prep_aux(qk_w, v_w, cpb_w1, cpb_b1, cpb_w2, sa_bias, mlp_w1, mlp_b1, mlp_w2, mlp_b2):
    scale = np.float32(1.0 / np.sqrt(QKD))
    qkwT = np.ascontiguousarray(qk_w.T).astype(np.float32).copy()   # (c, 2c)
    qkwT[:, :C] *= scale                                            # fold attn scale into q
    vwT = np.ascontiguousarray(v_w.T).astype(np.float32)            # (c, hid)

    bias = _log_cpb_np(HH, WW, cpb_w1, cpb_b1, cpb_w2)              # (heads, L, L)
    # multiplicative bias: exp(s+b) = exp(s)*exp(b); padded m-rows get 0 so
    # they vanish from the softmax sums and the attn@v contraction
    ebias = np.zeros((HEADS, LP, L), dtype=np.float32)
    ebias[:, :L, :] = np.exp(bias.transpose(0, 2, 1))               # [h, m, l]
    biasT_sb = np.empty((P, NCH, HEADS * L), dtype=np.float32)
    for ch in range(NCH):
        for h in range(HEADS):
            biasT_sb[:, ch, h * L:(h + 1) * L] = ebias[h, ch * P:(ch + 1) * P, :]

    w1T = np.empty((P, 2, HID), dtype=np.float32)                   # [p, kt, o]
    for kt in range(2):
        w1T[:, kt, :] = mlp_w1[:, kt * P:(kt + 1) * P].T
    w2T = np.empty((P, 2, OUT_CH), dtype=np.float32)
    for kt in range(2):
        w2T[:, kt, :] = mlp_w2[:, kt * P:(kt + 1) * P].T
    b1c = (mlp_w1 @ sa_bias.reshape(-1) + mlp_b1).astype(np.float32).reshape(2, P).T.copy()  # (128, 2)
    b2c = mlp_b2.astype(np.float32).reshape(P, 1).copy()
    return {
        "qkwT": qkwT.astype(np.float16),
        "vwT": vwT.astype(np.float16),
        "biasT": biasT_sb.astype(NPBF16),
        "ones32": np.ones((P, 32), dtype=NPBF16),
        "w1T": w1T.astype(np.float16),
        "w2T": w2T.astype(np.float16),
        "b1c": b1c, "b2c": b2c,
    }


def _gap(ap):
    """View a (128, 1024) psum tile as (128, 2, 288): pieces at [0:288], [512:800]."""
    return ap.rearrange("p (g c) -> p g c", c=512)[:, :, :PIECE]


def _pieces(ap576):
    """View a contiguous (128, 576) AP as (128, 2, 288)."""
    return ap576.rearrange("p (g c) -> p g c", c=PIECE)


def build_kernel(seqs=SEQ_PER_CORE, num_devices=NCORES):
    nc = bacc.Bacc("TRN2", target_bir_lowering=False, debug=False,
                   num_devices=num_devices)
    x_d = nc.dram_tensor("x", [seqs, C, L], F16, kind="ExternalInput").ap()
    qkwT_d = nc.dram_tensor("qkwT", [C, 2 * C], F16, kind="ExternalInput").ap()
    vwT_d = nc.dram_tensor("vwT", [C, HID], F16, kind="ExternalInput").ap()
    biasT_d = nc.dram_tensor("biasT", [P, NCH, HEADS * L], BF16, kind="ExternalInput").ap()
    ones32_d = nc.dram_tensor("ones32", [P, 32], BF16, kind="ExternalInput").ap()
    w1T_d = nc.dram_tensor("w1T", [P, 2, HID], F16, kind="ExternalInput").ap()
    w2T_d = nc.dram_tensor("w2T", [P, 2, OUT_CH], F16, kind="ExternalInput").ap()
    b1c_d = nc.dram_tensor("b1c", [P, 2], F32, kind="ExternalInput").ap()
    b2c_d = nc.dram_tensor("b2c", [P, 1], F32, kind="ExternalInput").ap()
    y_d = nc.dram_tensor("y", [seqs, OUT_CH, L], F16, kind="ExternalOutput").ap()

    EXP = mybir.ActivationFunctionType.Exp
    GELU = mybir.ActivationFunctionType.Gelu
    MULT = mybir.AluOpType.mult

    with tile.TileContext(nc) as tc:
        with (
            tc.tile_pool(name="const", bufs=1) as cpool,
            tc.tile_pool(name="xin", bufs=seqs) as xpool,
            tc.tile_pool(name="qk", bufs=3) as qkpool,
            tc.tile_pool(name="vt", bufs=3) as vtpool,
            tc.tile_pool(name="ex", bufs=3) as expool,
            tc.tile_pool(name="sm", bufs=2) as smpool,
            tc.tile_pool(name="xb", bufs=2) as xbpool,
            tc.tile_pool(name="drb", bufs=2, space="DRAM") as drpool,
            tc.tile_pool(name="acts", bufs=seqs) as apool,
            tc.tile_pool(name="mlp", bufs=2) as mpool,
            tc.tile_pool(name="psg", bufs=3, space="PSUM") as psg,
            tc.tile_pool(name="psb", bufs=1, space="PSUM") as psb,
        ):
            qkwT = cpool.tile([C, 2 * C], F16)
            vwT = cpool.tile([C, HID], F16)
            biasT = cpool.tile([P, NCH, HEADS * L], BF16)
            ones32 = cpool.tile([P, 32], BF16)
            w1T = cpool.tile([P, 2, HID], F16)
            w2T = cpool.tile([P, 2, OUT_CH], F16)
            b1c = cpool.tile([P, 2], F32)
            b2c = cpool.tile([P, 1], F32)
            nc.sync.dma_start(qkwT[:], qkwT_d[:])
            nc.sync.dma_start(vwT[:], vwT_d[:])
            # per-seq x tiles; bufs=seqs so all DMAs prefetch immediately and
            # the first matmul only waits for x[0]
            x_tiles = []
            for t in range(seqs):
                xt = xpool.tile([C, LP], F16)
                nc.sync.dma_start(xt[:, :L], x_d[t])
                nc.gpsimd.memset(xt[:, L:].bitcast(F32), 0.0)
                x_tiles.append(xt)
            for sb_t, dr in ((ones32, ones32_d), (biasT, biasT_d),
                             (w1T, w1T_d), (w2T, w2T_d),
                             (b1c, b1c_d), (b2c, b2c_d)):
                nc.sync.dma_start(sb_t[:], dr[:])

            _last_exp = [None]
            _dep_done = [False]
            a_tiles = {}
            ex_tiles = {}
            vt_tiles = {}

            # ---------------- Phase A: attention ----------------
            def front(t):
                """qk/v projections + scores/exp/bias for seq t."""
                xt = x_tiles[t]
                # qk projection: q rows (h0 d, h1 d), k rows (h0 d, h1 d)
                q_sb = qkpool.tile([P, L], F16, tag="q")
                k_sb = qkpool.tile([P, LP], F16, tag="k")
                for mt, dst in ((0, q_sb[:, :L]), (1, k_sb[:, :L])):
                    pqk = psg.tile([P, 1024], F32, tag="g")
                    for lh in range(2):
                        nc.tensor.matmul(
                            _gap(pqk[:])[:, lh],
                            qkwT[:, mt * P:(mt + 1) * P],
                            xt[:, lh * PIECE:(lh + 1) * PIECE],
                            start=True, stop=True)
                    nc.vector.tensor_copy(_pieces(dst), _gap(pqk[:]))
                nc.gpsimd.memset(k_sb[:, L:].bitcast(F32), 0.0)

                # vT: (m-part chunks, 256 = both heads' d); chunks 0-3 in one
                # psum tile + one batched copy, chunk 4 separate
                vt_sb = vtpool.tile([P, NCH, HID], F16)
                pv4 = psg.tile([P, 1024], F32, tag="g")
                for ch in range(4):
                    nc.tensor.matmul(
                        pv4[:].rearrange("p (c n) -> p c n", n=HID)[:, ch],
                        xt[:, ch * P:(ch + 1) * P], vwT[:],
                        start=True, stop=True)
                nc.vector.tensor_copy(
                    vt_sb[:, :4, :], pv4[:].rearrange("p (c n) -> p c n", n=HID))
                pv = psg.tile([P, 1024], F32, tag="g")
                nc.tensor.matmul(pv[:, :HID], xt[:, 4 * P:5 * P], vwT[:],
                                 start=True, stop=True)
                nc.vector.tensor_copy(vt_sb[:, 4, :], pv[:, :HID])
                vt_tiles[t] = vt_sb

                # scores + exp + bias-mult per (chunk, head)
                ex_sb = expool.tile([P, NCH, HEADS * L], BF16)
                for ch in range(NCH):
                    for h in range(HEADS):
                        # K=64 matmuls; the two heads run on disjoint PE
                        # row-groups into different PSUM banks (concurrent)
                        psc = psg.tile([P, 1024], F32, tag="g")
                        for lh in range(2):
                            nc.tensor.matmul(
                                _gap(psc[:])[:, lh],
                                k_sb[h * QKD:(h + 1) * QKD, ch * P:(ch + 1) * P],
                                q_sb[h * QKD:(h + 1) * QKD, lh * PIECE:(lh + 1) * PIECE],
                                start=True, stop=True)
                        _last_exp[0] = nc.scalar.activation(
                            _pieces(ex_sb[:, ch, h * L:(h + 1) * L]), _gap(psc[:]), EXP)
                        # engine-balance the bias multiply: DVE is 2x faster
                        # per element but DVE is the busier engine
                        eng = nc.gpsimd if (ch in (0, 2, 4)) else nc.vector
                        eng.tensor_tensor(
                            ex_sb[:, ch, h * L:(h + 1) * L],
                            ex_sb[:, ch, h * L:(h + 1) * L],
                            biasT[:, ch, h * L:(h + 1) * L], MULT)
                ex_tiles[t] = ex_sb

            def tail(t):
                """Dense (ungated) PE work for seq t: sums + attn@v, then the
                1/sums broadcast and the normalize. Emitted after front(t+1)
                so these matmuls fill the PE while seq t+1's scores trickle
                through the exp gate."""
                ex_sb = ex_tiles[t]
                vt_sb = vt_tiles[t]
                # softmax denominators first (the 1/sums DMA bounce then
                # overlaps the attn@v chains): two wide ones-matmul chains in
                # one psum tile, the 128-col remainder in a second, 32x32
                # transposes straight out of PSUM, sparse reciprocal,
                # transpose back, broadcast via DRAM bounce
                strans = smpool.tile([32, 4 * PIECE], F32, tag="strans")
                xb = xbpool.tile([P, 4 * PIECE], BF16)
                psA = psg.tile([P, 1024], F32, tag="g")
                for off in (0, 512):
                    for ch in range(NCH):
                        nc.tensor.matmul(
                            psA[0:32, off:off + 512], ones32[:],
                            ex_sb[:, ch, off:off + 512],
                            start=(ch == 0), stop=(ch == NCH - 1))
                for off in (0, 512):
                    nc.vector.transpose(strans[:, off:off + 512],
                                        psA[0:32, off:off + 512])
                psB = psg.tile([P, 1024], F32, tag="g")
                for ch in range(NCH):
                    nc.tensor.matmul(
                        psB[0:32, 0:128], ones32[:],
                        ex_sb[:, ch, 1024:1152],
                        start=(ch == 0), stop=(ch == NCH - 1))
                nc.vector.transpose(strans[:, 1024:1152], psB[0:32, 0:128])
                sparse = smpool.tile([32, 4 * PIECE], BF16, tag="sparse")
                nc.gpsimd.memset(sparse[:], 0.0)
                with nc.allow_low_precision(reason="softmax 1/sums in bf16 is within tolerance"):
                    nc.vector.reciprocal(
                        sparse[:].rearrange("p (b s) -> p b s", s=32)[:, :, 0],
                        strans[:].rearrange("p (b s) -> p b s", s=32)[:, :, 0])
                invrow = smpool.tile([32, 4 * PIECE], BF16, tag="invrow")
                nc.vector.transpose(invrow[:], sparse[:])
                inv_dr = drpool.tile([1, 4 * PIECE], BF16)
                nc.sync.dma_start(inv_dr[:], invrow[0:1, :])
                nc.sync.dma_start(xb[:], inv_dr[:].to_broadcast((P, 4 * PIECE)))

                # out = vT.T @ expT: dense accumulation chains, all inputs ready
                a_sb = apool.tile([P, 2, L], F16)
                for h in range(HEADS):
                    pm = psg.tile([P, 1024], F32, tag="g")
                    for ch in range(NCH):
                        for lh in range(2):  # reuse loaded vT weights across pieces
                            nc.tensor.matmul(
                                _gap(pm[:])[:, lh],
                                vt_sb[:, ch, h * VD:(h + 1) * VD],
                                ex_sb[:, ch, h * L + lh * PIECE: h * L + (lh + 1) * PIECE],
                                start=(ch == 0), stop=(ch == NCH - 1))
                    nc.vector.tensor_tensor(
                        _pieces(a_sb[:, h, :]), _gap(pm[:]),
                        _pieces(xb[:, h * 2 * PIECE:(h * 2 + 2) * PIECE]), MULT)
                a_tiles[t] = a_sb

            for t in range(seqs):
                front(t)
                if t >= 1:
                    tail(t - 1)
            tail(seqs - 1)

            # ---------------- Phase B: MLP ----------------
            for t in range(seqs):
                a_sb = a_tiles[t]
                g_sb = mpool.tile([P, 2, L], F16, tag="g")
                for mt in range(2):
                    py1 = psb.tile([P, 1024], F32, tag="b")
                    for lh in range(2):
                        for kt in range(2):
                            nc.tensor.matmul(
                                _gap(py1[:])[:, lh],
                                w1T[:, kt, mt * P:(mt + 1) * P],
                                a_sb[:, kt, lh * PIECE:(lh + 1) * PIECE],
                                start=(kt == 0), stop=(kt == 1))
                    _g = nc.scalar.activation(
                        _pieces(g_sb[:, mt, :]), _gap(py1[:]), GELU,
                        bias=b1c[:, mt:mt + 1], scale=1.0)
                    if not _dep_done[0] and _last_exp[0] is not None:
                        # keep ACT exp->gelu strictly phase-ordered: the two live in
                        # different ACT table sets, and a mid-phase switch costs ~2.7us
                        add_dep_helper(_last_exp[0].ins, _g.ins, sync=False,
                                       reason="ACT table set phase order")
                        _dep_done[0] = True
                y_sb = mpool.tile([OUT_CH, L], F16, tag="y")
                py2 = psb.tile([P, 1024], F32, tag="b")
                for lh in range(2):
                    for kt in range(2):
                        nc.tensor.matmul(
                            _gap(py2[:])[:, lh], w2T[:, kt, :],
                            g_sb[:, kt, lh * PIECE:(lh + 1) * PIECE],
                            start=(kt == 0), stop=(kt == 1))
                nc.vector.tensor_scalar_add(
                    _pieces(y_sb[:]), _gap(py2[:]), b2c[:, 0:1])
                nc.sync.dma_start(y_d[t], y_sb[:])
    nc.compile()
    return nc


_CACHED = {}


def _get_nc():
    if "nc" not in _CACHED:
        _CACHED["nc"] = build_kernel()
    return _CACHED["nc"]


def make_in_maps(x, aux):
    xr = np.ascontiguousarray(x.reshape(NSEQ, C, L)).astype(np.float16)
    in_maps = []
    for i in range(NCORES):
        m = {"x": xr[i * SEQ_PER_CORE:(i + 1) * SEQ_PER_CORE]}
        m.update(aux)
        in_maps.append(m)
    return in_maps


def kernel(x, qk_w, v_w, cpb_w1, cpb_b1, cpb_w2, sa_bias,
           mlp_w1, mlp_b1, mlp_w2, mlp_b2):
    x = np.asarray(x)
    aux = _prep_aux(np.asarray(qk_w), np.asarray(v_w), np.asarray(cpb_w1),
                    np.asarray(cpb_b1), np.asarray(cpb_w2), np.asarray(sa_bias),
                    np.asarray(mlp_w1), np.asarray(mlp_b1), np.asarray(mlp_w2),
                    np.asarray(mlp_b2))
    nc = _get_nc()
    in_maps = make_in_maps(x, aux)
    res = run_bass_kernel_spmd(nc, in_maps, core_ids=list(range(NCORES)))
    y = np.concatenate([res.results[i]["y"] for i in range(NCORES)], axis=0)
    return y.astype(np.float32).reshape(B, S, OUT_CH, HH, WW)


if __name__ == "__main__":
    import reference
    inputs = reference.setup_inputs()
    inputs = {k: np.asarray(v) for k, v in inputs.items()}
    out = kernel(**inputs)
    exp = np.asarray(reference.reference(**reference.setup_inputs()))
    err = np.abs(out - exp).max() / np.abs(exp).max()
    print("Relative error:", err)


# revision 9
# speedup vs baseline: 1.0262x; 1.0262x over previous
"""Trainium2 Bass kernel for nn_AttentionNeuralOperator (dense_transformer), v2.

Strategy (8 NeuronCores, data-parallel over the b*s=64 sequences, 8 per core):
  All matmuls in 16-bit (fp16 front-end / bf16 post-exp) with fp32 PSUM:
  fp16 has tf32-class mantissa so scores keep precision; E=exp(s)*exp(b) is
  bf16 for range. LDWEIGHTS runs 4x faster than fp32 (FWL), DMA bytes halve.
  Per sequence (c=128, L=576, 2 heads, d_qk=64, d_v=128):
    qk  = qk_wT.T @ x            -> q,k (d, L) fp16        [PE] -> DVE copies
    vT  = x.T @ v_wT             -> v (m, d) fp16          [PE]
    scoresT = k_h.T q_h          -> (m-part, l-free)       [PE, heads on
                                    disjoint row-groups, concurrent]
    expT = exp(scoresT) -> bf16  [ACT], multiplied by host-precomputed
      exp(bias) table (bf16), split DVE/GpSimd for engine balance.
    sums via ones-matmul, reciprocal via 32x32 transpose trick (bf16),
      broadcast via DRAM bounce.
    out_h = vT_h.T @ expT_h, normalized by 1/sums           [PE + DVE]
    1x1-conv MLP (fp16 weights) with exact GELU             [PE + ACT]
  y is fp16 on device; host upcasts to fp32.
"""
import sys
sys.path.insert(0, "/opt/trn_rl_repo")
import numpy as np
import ml_dtypes

import concourse.bass as bass
import concourse.tile as tile
from concourse.tile import add_dep_helper
from concourse import bacc, mybir
from concourse.bass_utils import run_bass_kernel_spmd

P = 128
HEADS = 2
B, S, C, HH, WW = 2, 32, 128, 24, 24
L = HH * WW            # 576
LP = 640               # m padded to 5*128
NSEQ = B * S           # 64
NCORES = 8
SEQ_PER_CORE = NSEQ // NCORES  # 8
HID = 256
OUT_CH = 128
QKD = C // HEADS       # 64
VD = HID // HEADS      # 128
NCH = LP // P          # 5 m-chunks
PIECE = 288            # l-piece
F32 = mybir.dt.float32
F16 = mybir.dt.float16
BF16 = mybir.dt.bfloat16
NPBF16 = ml_dtypes.bfloat16


def _log_cpb_np(h, w, w1, b1, w2):
    """Host fp32 mirror of the reference CPB MLP, on the 47x47 delta grid."""
    dy = np.arange(-(h - 1), h, dtype=np.float32)
    dx = np.arange(-(w - 1), w, dtype=np.float32)
    rel = np.stack(np.meshgrid(dy, dx, indexing="ij"), axis=-1)     # (2h-1, 2w-1, 2)
    denom = np.array([max(h - 1, 1), max(w - 1, 1)], dtype=np.float32)
    rel = rel / denom * np.float32(8.0)
    rel = np.sign(rel) * np.log2(np.float32(1.0) + np.abs(rel)) / np.float32(np.log2(8.0))
    hid_act = np.maximum(rel @ w1.T + b1, np.float32(0.0))          # (2h-1, 2w-1, c)
    tab = (hid_act @ w2.T).astype(np.float32)                       # (2h-1, 2w-1, heads)
    yl = np.repeat(np.arange(h), w)
    xl = np.tile(np.arange(w), h)
    DY = yl[:, None] - yl[None, :] + (h - 1)                        # (L, L)
    DX = xl[:, None] - xl[None, :] + (w - 1)
    return tab[DY, DX].transpose(2, 0, 1)                           # (heads, L, L)


def _prep_aux(qk_w, v_w, cpb_w1, cpb_b1, cpb_w2, sa_bias, mlp_w1, mlp_b1, mlp_w2, mlp_b2):
    scale = np.float32(1.0 / np.sqrt(QKD))
    qkwT = np.ascontiguousarray(qk_w.T).astype(np.float32).copy()   # (c, 2c)
    qkwT[:, :C] *= scale                                            # fold attn scale into q
    vwT = np.ascontiguousarray(v_w.T).astype(np.float32)            # (c, hid)

    bias = _log_cpb_np(HH, WW, cpb_w1, cpb_b1, cpb_w2)              # (heads, L, L)
    # multiplicative bias: exp(s+b) = exp(s)*exp(b); padded m-rows get 0 so
    # they vanish from the softmax sums and the attn@v contraction
    ebias = np.zeros((HEADS, LP, L), dtype=np.float32)
    ebias[:, :L, :] = np.exp(bias.transpose(0, 2, 1))               # [h, m, l]
    biasT_sb = np.empty((P, NCH, HEADS * L), dtype=np.float32)
    for ch in range(NCH):
        for h in range(HEADS):
            biasT_sb[:, ch, h * L:(h + 1) * L] = ebias[h, ch * P:(ch + 1) * P, :]

    w1T = np.empty((P, 2, HID), dtype=np.float32)                   # [p, kt, o]
    for kt in range(2):
        w1T[:, kt, :] = mlp_w1[:, kt * P:(kt + 1) * P].T
    w2T = np.empty((P, 2, OUT_CH), dtype=np.float32)
    for kt in range(2):
        w2T[:, kt, :] = mlp_w2[:, kt * P:(kt + 1) * P].T
    b1c = (mlp_w1 @ sa_bias.reshape(-1) + mlp_b1).astype(np.float32).reshape(2, P).T.copy()  # (128, 2)
    b2c = mlp_b2.astype(np.float32).reshape(P, 1).copy()
    return {
        "qkwT": qkwT.astype(np.float16),
        "vwT": vwT.astype(np.float16),
        "biasT": biasT_sb.astype(NPBF16),
        "ones32": np.ones((P, 32), dtype=NPBF16),
        "w1T": w1T.astype(np.float16),
        "w2T": w2T.astype(np.float16),
        "b1c": b1c, "b2c": b2c,
    }


def _gap(ap):
    """View a (128, 1024) psum tile as (128, 2, 288): pieces at [0:288], [512:800]."""
    return ap.rearrange("p (g c) -> p g c", c=512)[:, :, :PIECE]


def _pieces(ap576):
    """View a contiguous (128, 576) AP as (128, 2, 288)."""
    return ap576.rearrange("p (g c) -> p g c", c=PIECE)


def build_kernel(seqs=SEQ_PER_CORE, num_devices=NCORES):
    nc = bacc.Bacc("TRN2", target_bir_lowering=False, debug=False,
                   num_devices=num_devices)
    x_d = nc.dram_tensor("x", [seqs, C, L], F16, kind="ExternalInput").ap()
    qkwT_d = nc.dram_tensor("qkwT", [C, 2 * C], F16, kind="ExternalInput").ap()
    vwT_d = nc.dram_tensor("vwT", [C, HID], F16, kind="ExternalInput").ap()
    biasT_d = nc.dram_tensor("biasT", [P, NCH, HEADS * L], BF16, kind="ExternalInput").ap()
    ones32_d = nc.dram_tensor("ones32", [P, 32], BF16, kind="ExternalInput").ap()
    w1T_d = nc.dram_tensor("w1T", [P, 2, HID], F16, kind="ExternalInput").ap()
    w2T_d = nc.dram_tensor("w2T", [P, 2, OUT_CH], F16, kind="ExternalInput").ap()
    b1c_d = nc.dram_tensor("b1c", [P, 2], F32, kind="ExternalInput").ap()
    b2c_d = nc.dram_tensor("b2c", [P, 1], F32, kind="ExternalInput").ap()
    y_d = nc.dram_tensor("y", [seqs, OUT_CH, L], F16, kind="ExternalOutput").ap()

    EXP = mybir.ActivationFunctionType.Exp
    GELU = mybir.ActivationFunctionType.Gelu
    MULT = mybir.AluOpType.mult

    with tile.TileContext(nc) as tc:
        with (
            tc.tile_pool(name="const", bufs=1) as cpool,
            tc.tile_pool(name="xin", bufs=seqs) as xpool,
            tc.tile_pool(name="qk", bufs=3) as qkpool,
            tc.tile_pool(name="vt", bufs=3) as vtpool,
            tc.tile_pool(name="ex", bufs=3) as expool,
            tc.tile_pool(name="sm", bufs=2) as smpool,
            tc.tile_pool(name="xb", bufs=2) as xbpool,
            tc.tile_pool(name="drb", bufs=2, space="DRAM") as drpool,
            tc.tile_pool(name="acts", bufs=seqs) as apool,
            tc.tile_pool(name="mlp", bufs=2) as mpool,
            tc.tile_pool(name="psg", bufs=3, space="PSUM") as psg,
            tc.tile_pool(name="psb", bufs=1, space="PSUM") as psb,
        ):
            qkwT = cpool.tile([C, 2 * C], F16)
            vwT = cpool.tile([C, HID], F16)
            biasT = cpool.tile([P, NCH, HEADS * L], BF16)
            ones32 = cpool.tile([P, 32], BF16)
            w1T = cpool.tile([P, 2, HID], F16)
            w2T = cpool.tile([P, 2, OUT_CH], F16)
            b1c = cpool.tile([P, 2], F32)
            b2c = cpool.tile([P, 1], F32)
            nc.sync.dma_start(qkwT[:], qkwT_d[:])
            nc.sync.dma_start(vwT[:], vwT_d[:])
            # per-seq x tiles; bufs=seqs so all DMAs prefetch immediately and
            # the first matmul only waits for x[0]
            x_tiles = []
            for t in range(seqs):
                xt = xpool.tile([C, LP], F16)
                nc.sync.dma_start(xt[:, :L], x_d[t])
                nc.gpsimd.memset(xt[:, L:].bitcast(F32), 0.0)
                x_tiles.append(xt)
            for sb_t, dr in ((ones32, ones32_d), (biasT, biasT_d),
                             (w1T, w1T_d), (w2T, w2T_d),
                             (b1c, b1c_d), (b2c, b2c_d)):
                nc.sync.dma_start(sb_t[:], dr[:])

            _last_exp = [None]
            _dep_done = [False]
            a_tiles = {}
            ex_tiles = {}
            vt_tiles = {}

            def mm_reuse(ms):
                """Matmuls sharing one stationary operand: only the first
                emits LDWEIGHTS; the follow-ons stream back-to-back over the
                already-loaded weights. Correct only while the scheduler keeps
                the pair adjacent in the PE stream (verified by the HW
                rel-err check)."""
                for i, (out, lhsT, rhs, st, sp) in enumerate(ms):
                    mm = nc.tensor.matmul(out, lhsT, rhs, start=st, stop=sp)
                    if i > 0:
                        mm.ins.ldweights = False

            # ---------------- Phase A: attention ----------------
            def front(t):
                """qk/v projections + scores/exp/bias for seq t."""
                xt = x_tiles[t]
                # qk projection: q rows (h0 d, h1 d), k rows (h0 d, h1 d)
                q_sb = qkpool.tile([P, L], F16, tag="q")
                k_sb = qkpool.tile([P, LP], F16, tag="k")
                for mt, dst in ((0, q_sb[:, :L]), (1, k_sb[:, :L])):
                    pqk = psg.tile([P, 1024], F32, tag="g")
                    mm_reuse([(_gap(pqk[:])[:, lh],
                               qkwT[:, mt * P:(mt + 1) * P],
                               xt[:, lh * PIECE:(lh + 1) * PIECE],
                               True, True) for lh in range(2)])
                    nc.vector.tensor_copy(_pieces(dst), _gap(pqk[:]))
                nc.gpsimd.memset(k_sb[:, L:].bitcast(F32), 0.0)

                # vT: (m-part chunks, 256 = both heads' d); chunks 0-3 in one
                # psum tile + one batched copy, chunk 4 separate
                vt_sb = vtpool.tile([P, NCH, HID], F16)
                pv4 = psg.tile([P, 1024], F32, tag="g")
                for ch in range(4):
                    nc.tensor.matmul(
                        pv4[:].rearrange("p (c n) -> p c n", n=HID)[:, ch],
                        xt[:, ch * P:(ch + 1) * P], vwT[:],
                        start=True, stop=True)
                nc.vector.tensor_copy(
                    vt_sb[:, :4, :], pv4[:].rearrange("p (c n) -> p c n", n=HID))
                pv = psg.tile([P, 1024], F32, tag="g")
                nc.tensor.matmul(pv[:, :HID], xt[:, 4 * P:5 * P], vwT[:],
                                 start=True, stop=True)
                nc.vector.tensor_copy(vt_sb[:, 4, :], pv[:, :HID])
                vt_tiles[t] = vt_sb

                # scores + exp + bias-mult per (chunk, head)
                ex_sb = expool.tile([P, NCH, HEADS * L], BF16)
                for ch in range(NCH):
                    for h in range(HEADS):
                        # K=64 matmuls; the two heads run on disjoint PE
                        # row-groups into different PSUM banks (concurrent)
                        psc = psg.tile([P, 1024], F32, tag="g")
                        mm_reuse([(_gap(psc[:])[:, lh],
                                   k_sb[h * QKD:(h + 1) * QKD, ch * P:(ch + 1) * P],
                                   q_sb[h * QKD:(h + 1) * QKD, lh * PIECE:(lh + 1) * PIECE],
                                   True, True) for lh in range(2)])
                        _last_exp[0] = nc.scalar.activation(
                            _pieces(ex_sb[:, ch, h * L:(h + 1) * L]), _gap(psc[:]), EXP)
                        # engine-balance the bias multiply: DVE is 2x faster
                        # per element but DVE is the busier engine
                        eng = nc.gpsimd if (ch in (0, 2, 4)) else nc.vector
                        eng.tensor_tensor(
                            ex_sb[:, ch, h * L:(h + 1) * L],
                            ex_sb[:, ch, h * L:(h + 1) * L],
                            biasT[:, ch, h * L:(h + 1) * L], MULT)
                ex_tiles[t] = ex_sb

            def tail(t):
                """Dense (ungated) PE work for seq t: sums + attn@v, then the
                1/sums broadcast and the normalize. Emitted after front(t+1)
                so these matmuls fill the PE while seq t+1's scores trickle
                through the exp gate."""
                ex_sb = ex_tiles[t]
                vt_sb = vt_tiles[t]
                # softmax denominators first (the 1/sums DMA bounce then
                # overlaps the attn@v chains): two wide ones-matmul chains in
                # one psum tile, the 128-col remainder in a second, 32x32
                # transposes straight out of PSUM, sparse reciprocal,
                # transpose back, broadcast via DRAM bounce
                strans = smpool.tile([32, 4 * PIECE], F32, tag="strans")
                xb = xbpool.tile([P, 4 * PIECE], BF16)
                psA = psg.tile([P, 1024], F32, tag="g")
                for off in (0, 512):
                    for ch in range(NCH):
                        nc.tensor.matmul(
                            psA[0:32, off:off + 512], ones32[:],
                            ex_sb[:, ch, off:off + 512],
                            start=(ch == 0), stop=(ch == NCH - 1))
                for off in (0, 512):
                    nc.vector.transpose(strans[:, off:off + 512],
                                        psA[0:32, off:off + 512])
                psB = psg.tile([P, 1024], F32, tag="g")
                for ch in range(NCH):
                    nc.tensor.matmul(
                        psB[0:32, 0:128], ones32[:],
                        ex_sb[:, ch, 1024:1152],
                        start=(ch == 0), stop=(ch == NCH - 1))
                nc.vector.transpose(strans[:, 1024:1152], psB[0:32, 0:128])
                sparse = smpool.tile([32, 4 * PIECE], BF16, tag="sparse")
                nc.gpsimd.memset(sparse[:], 0.0)
                with nc.allow_low_precision(reason="softmax 1/sums in bf16 is within tolerance"):
                    nc.vector.reciprocal(
                        sparse[:].rearrange("p (b s) -> p b s", s=32)[:, :, 0],
                        strans[:].rearrange("p (b s) -> p b s", s=32)[:, :, 0])
                invrow = smpool.tile([32, 4 * PIECE], BF16, tag="invrow")
                nc.vector.transpose(invrow[:], sparse[:])
                inv_dr = drpool.tile([1, 4 * PIECE], BF16)
                nc.sync.dma_start(inv_dr[:], invrow[0:1, :])
                nc.sync.dma_start(xb[:], inv_dr[:].to_broadcast((P, 4 * PIECE)))

                # out = vT.T @ expT: dense accumulation chains, all inputs ready
                a_sb = apool.tile([P, 2, L], F16)
                for h in range(HEADS):
                    pm = psg.tile([P, 1024], F32, tag="g")
                    for ch in range(NCH):
                        mm_reuse([(_gap(pm[:])[:, lh],
                                   vt_sb[:, ch, h * VD:(h + 1) * VD],
                                   ex_sb[:, ch, h * L + lh * PIECE: h * L + (lh + 1) * PIECE],
                                   ch == 0, ch == NCH - 1) for lh in range(2)])
                    nc.vector.tensor_tensor(
                        _pieces(a_sb[:, h, :]), _gap(pm[:]),
                        _pieces(xb[:, h * 2 * PIECE:(h * 2 + 2) * PIECE]), MULT)
                a_tiles[t] = a_sb

            for t in range(seqs):
                front(t)
                if t >= 1:
                    tail(t - 1)
            tail(seqs - 1)

            # ---------------- Phase B: MLP ----------------
            for t in range(seqs):
                a_sb = a_tiles[t]
                g_sb = mpool.tile([P, 2, L], F16, tag="g")
                for mt in range(2):
                    py1 = psb.tile([P, 1024], F32, tag="b")
                    for kt in range(2):
                        mm_reuse([(_gap(py1[:])[:, lh],
                                   w1T[:, kt, mt * P:(mt + 1) * P],
                                   a_sb[:, kt, lh * PIECE:(lh + 1) * PIECE],
                                   kt == 0, kt == 1) for lh in range(2)])
                    _g = nc.scalar.activation(
                        _pieces(g_sb[:, mt, :]), _gap(py1[:]), GELU,
                        bias=b1c[:, mt:mt + 1], scale=1.0)
                    if not _dep_done[0] and _last_exp[0] is not None:
                        # keep ACT exp->gelu strictly phase-ordered: the two live in
                        # different ACT table sets, and a mid-phase switch costs ~2.7us
                        add_dep_helper(_last_exp[0].ins, _g.ins, sync=False,
                                       reason="ACT table set phase order")
                        _dep_done[0] = True
                y_sb = mpool.tile([OUT_CH, L], F16, tag="y")
                py2 = psb.tile([P, 1024], F32, tag="b")
                for kt in range(2):
                    mm_reuse([(_gap(py2[:])[:, lh], w2T[:, kt, :],
                               g_sb[:, kt, lh * PIECE:(lh + 1) * PIECE],
                               kt == 0, kt == 1) for lh in range(2)])
                nc.vector.tensor_scalar_add(
                    _pieces(y_sb[:]), _gap(py2[:]), b2c[:, 0:1])
                nc.sync.dma_start(y_d[t], y_sb[:])
    nc.compile()
    return nc


_CACHED = {}


def _get_nc():
    if "nc" not in _CACHED:
        _CACHED["nc"] = build_kernel()
    return _CACHED["nc"]


def make_in_maps(x, aux):
    xr = np.ascontiguousarray(x.reshape(NSEQ, C, L)).astype(np.float16)
    in_maps = []
    for i in range(NCORES):
        m = {"x": xr[i * SEQ_PER_CORE:(i + 1) * SEQ_PER_CORE]}
        m.update(aux)
        in_maps.append(m)
    return in_maps


def kernel(x, qk_w, v_w, cpb_w1, cpb_b1, cpb_w2, sa_bias,
           mlp_w1, mlp_b1, mlp_w2, mlp_b2):
    x = np.asarray(x)
    aux = _prep_aux(np.asarray(qk_w), np.asarray(v_w), np.asarray(cpb_w1),
                    np.asarray(cpb_b1), np.asarray(cpb_w2), np.asarray(sa_bias),
                    np.asarray(mlp_w1), np.asarray(mlp_b1), np.asarray(mlp_w2),
                    np.asarray(mlp_b2))
    nc = _get_nc()
    in_maps = make_in_maps(x, aux)
    res = run_bass_kernel_spmd(nc, in_maps, core_ids=list(range(NCORES)))
    y = np.concatenate([res.results[i]["y"] for i in range(NCORES)], axis=0)
    return y.astype(np.float32).reshape(B, S, OUT_CH, HH, WW)


if __name__ == "__main__":
    import reference
    inputs = reference.setup_inputs()
    inputs = {k: np.asarray(v) for k, v in inputs.items()}
    out = kernel(**inputs)
    exp = np.asarray(reference.reference(**reference.setup_inputs()))
    err = np.abs(out - exp).max() / np.abs(exp).max()
    print("Relative error:", err)


# revision 10
# speedup vs baseline: 1.1895x; 1.1590x over previous
"""Trainium2 Bass kernel for nn_AttentionNeuralOperator (dense_transformer), v2.

Strategy (8 NeuronCores, data-parallel over the b*s=64 sequences, 8 per core):
  All matmuls in 16-bit (fp16 front-end / bf16 post-exp) with fp32 PSUM:
  fp16 has tf32-class mantissa so scores keep precision; E=exp(s)*exp(b) is
  bf16 for range. LDWEIGHTS runs 4x faster than fp32 (FWL), DMA bytes halve.
  Per sequence (c=128, L=576, 2 heads, d_qk=64, d_v=128):
    qk  = qk_wT.T @ x            -> q,k (d, L) fp16        [PE] -> DVE copies
    vT  = x.T @ v_wT             -> v (m, d) fp16          [PE]
    scoresT = k_h.T q_h          -> (m-part, l-free)       [PE, heads on
                                    disjoint row-groups, concurrent]
    expT = exp(scoresT) -> bf16  [ACT], multiplied by host-precomputed
      exp(bias) table (bf16), split DVE/GpSimd for engine balance.
    sums via ones-matmul, reciprocal via 32x32 transpose trick (bf16),
      broadcast via DRAM bounce.
    out_h = vT_h.T @ expT_h, normalized by 1/sums           [PE + DVE]
    1x1-conv MLP (fp16 weights) with exact GELU             [PE + ACT]
  y is fp16 on device; host upcasts to fp32.
"""
import sys
sys.path.insert(0, "/opt/trn_rl_repo")
import numpy as np
import ml_dtypes

import concourse.bass as bass
import concourse.tile as tile
from concourse.tile import add_dep_helper
from concourse import bacc, mybir
from concourse.bass_utils import run_bass_kernel_spmd

P = 128
HEADS = 2
B, S, C, HH, WW = 2, 32, 128, 24, 24
L = HH * WW            # 576
LP = 640               # m padded to 5*128
NSEQ = B * S           # 64
NCORES = 8
SEQ_PER_CORE = NSEQ // NCORES  # 8
HID = 256
OUT_CH = 128
QKD = C // HEADS       # 64
VD = HID // HEADS      # 128
NCH = LP // P          # 5 m-chunks
PIECE = 288            # l-piece
F32 = mybir.dt.float32
F16 = mybir.dt.float16
BF16 = mybir.dt.bfloat16
NPBF16 = ml_dtypes.bfloat16


def _log_cpb_np(h, w, w1, b1, w2):
    """Host fp32 mirror of the reference CPB MLP, on the 47x47 delta grid."""
    dy = np.arange(-(h - 1), h, dtype=np.float32)
    dx = np.arange(-(w - 1), w, dtype=np.float32)
    rel = np.stack(np.meshgrid(dy, dx, indexing="ij"), axis=-1)     # (2h-1, 2w-1, 2)
    denom = np.array([max(h - 1, 1), max(w - 1, 1)], dtype=np.float32)
    rel = rel / denom * np.float32(8.0)
    rel = np.sign(rel) * np.log2(np.float32(1.0) + np.abs(rel)) / np.float32(np.log2(8.0))
    hid_act = np.maximum(rel @ w1.T + b1, np.float32(0.0))          # (2h-1, 2w-1, c)
    tab = (hid_act @ w2.T).astype(np.float32)                       # (2h-1, 2w-1, heads)
    yl = np.repeat(np.arange(h), w)
    xl = np.tile(np.arange(w), h)
    DY = yl[:, None] - yl[None, :] + (h - 1)                        # (L, L)
    DX = xl[:, None] - xl[None, :] + (w - 1)
    return tab[DY, DX].transpose(2, 0, 1)                           # (heads, L, L)


def _prep_aux(qk_w, v_w, cpb_w1, cpb_b1, cpb_w2, sa_bias, mlp_w1, mlp_b1, mlp_w2, mlp_b2):
    scale = np.float32(1.0 / np.sqrt(QKD))
    qkwT = np.ascontiguousarray(qk_w.T).astype(np.float32).copy()   # (c, 2c)
    qkwT[:, :C] *= scale                                            # fold attn scale into q
    vwT = np.ascontiguousarray(v_w.T).astype(np.float32)            # (c, hid)

    bias = _log_cpb_np(HH, WW, cpb_w1, cpb_b1, cpb_w2)              # (heads, L, L)
    # multiplicative bias: exp(s+b) = exp(s)*exp(b); padded m-rows get 0 so
    # they vanish from the softmax sums and the attn@v contraction
    ebias = np.zeros((HEADS, LP, L), dtype=np.float32)
    ebias[:, :L, :] = np.exp(bias.transpose(0, 2, 1))               # [h, m, l]
    biasT_sb = np.empty((P, NCH, HEADS * L), dtype=np.float32)
    for ch in range(NCH):
        for h in range(HEADS):
            biasT_sb[:, ch, h * L:(h + 1) * L] = ebias[h, ch * P:(ch + 1) * P, :]

    w1T = np.empty((P, 2, HID), dtype=np.float32)                   # [p, kt, o]
    for kt in range(2):
        w1T[:, kt, :] = mlp_w1[:, kt * P:(kt + 1) * P].T
    w2T = np.empty((P, 2, OUT_CH), dtype=np.float32)
    for kt in range(2):
        w2T[:, kt, :] = mlp_w2[:, kt * P:(kt + 1) * P].T
    b1c = (mlp_w1 @ sa_bias.reshape(-1) + mlp_b1).astype(np.float32).reshape(2, P).T.copy()  # (128, 2)
    b2c = mlp_b2.astype(np.float32).reshape(P, 1).copy()
    return {
        "qkwT": qkwT.astype(np.float16),
        "vwT": vwT.astype(np.float16),
        "biasT": biasT_sb.astype(NPBF16),
        "ones32": np.ones((P, 32), dtype=NPBF16),
        "w1T": w1T.astype(np.float16),
        "w2T": w2T.astype(np.float16),
        "b1c": b1c, "b2c": b2c,
    }


def _gap(ap):
    """View a (128, 1024) psum tile as (128, 2, 288): pieces at [0:288], [512:800]."""
    return ap.rearrange("p (g c) -> p g c", c=512)[:, :, :PIECE]


def _pieces(ap576):
    """View a contiguous (128, 576) AP as (128, 2, 288)."""
    return ap576.rearrange("p (g c) -> p g c", c=PIECE)


def build_kernel(seqs=SEQ_PER_CORE, num_devices=NCORES):
    nc = bacc.Bacc("TRN2", target_bir_lowering=False, debug=False,
                   num_devices=num_devices)
    x_d = nc.dram_tensor("x", [seqs, C, L], F16, kind="ExternalInput").ap()
    qkwT_d = nc.dram_tensor("qkwT", [C, 2 * C], F16, kind="ExternalInput").ap()
    vwT_d = nc.dram_tensor("vwT", [C, HID], F16, kind="ExternalInput").ap()
    biasT_d = nc.dram_tensor("biasT", [P, NCH, HEADS * L], BF16, kind="ExternalInput").ap()
    ones32_d = nc.dram_tensor("ones32", [P, 32], BF16, kind="ExternalInput").ap()
    w1T_d = nc.dram_tensor("w1T", [P, 2, HID], F16, kind="ExternalInput").ap()
    w2T_d = nc.dram_tensor("w2T", [P, 2, OUT_CH], F16, kind="ExternalInput").ap()
    b1c_d = nc.dram_tensor("b1c", [P, 2], F32, kind="ExternalInput").ap()
    b2c_d = nc.dram_tensor("b2c", [P, 1], F32, kind="ExternalInput").ap()
    y_d = nc.dram_tensor("y", [seqs, OUT_CH, L], F16, kind="ExternalOutput").ap()

    EXP = mybir.ActivationFunctionType.Exp
    GELU = mybir.ActivationFunctionType.Gelu
    MULT = mybir.AluOpType.mult

    with tile.TileContext(nc) as tc:
        with (
            tc.tile_pool(name="const", bufs=1) as cpool,
            tc.tile_pool(name="xin", bufs=seqs) as xpool,
            tc.tile_pool(name="qk", bufs=3) as qkpool,
            tc.tile_pool(name="vt", bufs=3) as vtpool,
            tc.tile_pool(name="ex", bufs=3) as expool,
            tc.tile_pool(name="sm", bufs=2) as smpool,
            tc.tile_pool(name="xb", bufs=2) as xbpool,
            tc.tile_pool(name="drb", bufs=2, space="DRAM") as drpool,
            tc.tile_pool(name="acts", bufs=seqs) as apool,
            tc.tile_pool(name="mlp", bufs=2) as mpool,
            tc.tile_pool(name="psg", bufs=3, space="PSUM") as psg,
            tc.tile_pool(name="psb", bufs=1, space="PSUM") as psb,
        ):
            qkwT = cpool.tile([C, 2 * C], F16)
            vwT = cpool.tile([C, HID], F16)
            biasT = cpool.tile([P, NCH, HEADS * L], BF16)
            ones32 = cpool.tile([P, 32], BF16)
            w1T = cpool.tile([P, 2, HID], F16)
            w2T = cpool.tile([P, 2, OUT_CH], F16)
            b1c = cpool.tile([P, 2], F32)
            b2c = cpool.tile([P, 1], F32)
            nc.sync.dma_start(qkwT[:], qkwT_d[:])
            nc.sync.dma_start(vwT[:], vwT_d[:])
            # per-seq x tiles; bufs=seqs so all DMAs prefetch immediately and
            # the first matmul only waits for x[0]
            x_tiles = []
            for t in range(seqs):
                xt = xpool.tile([C, LP], F16)
                nc.sync.dma_start(xt[:, :L], x_d[t])
                nc.gpsimd.memset(xt[:, L:].bitcast(F32), 0.0)
                x_tiles.append(xt)
            for sb_t, dr in ((ones32, ones32_d), (biasT, biasT_d),
                             (w1T, w1T_d), (w2T, w2T_d),
                             (b1c, b1c_d), (b2c, b2c_d)):
                nc.sync.dma_start(sb_t[:], dr[:])

            _last_exp = [None]
            _dep_done = [False]
            a_tiles = {}
            ex_tiles = {}
            vt_tiles = {}

            # ---------------- Phase A: attention ----------------
            def front(t):
                """qk/v projections + scores/exp/bias for seq t."""
                xt = x_tiles[t]
                # qk projection: q rows (h0 d, h1 d), k rows (h0 d, h1 d)
                q_sb = qkpool.tile([P, L], F16, tag="q")
                k_sb = qkpool.tile([P, LP], F16, tag="k")
                for mt, dst in ((0, q_sb[:, :L]), (1, k_sb[:, :L])):
                    pqk = psg.tile([P, 1024], F32, tag="g")
                    for lh in range(2):
                        nc.tensor.matmul(
                            _gap(pqk[:])[:, lh],
                            qkwT[:, mt * P:(mt + 1) * P],
                            xt[:, lh * PIECE:(lh + 1) * PIECE],
                            start=True, stop=True)
                    nc.vector.tensor_copy(_pieces(dst), _gap(pqk[:]))
                nc.gpsimd.memset(k_sb[:, L:].bitcast(F32), 0.0)

                # vT: (m-part chunks, 256 = both heads' d); chunks 0-3 in one
                # psum tile + one batched copy, chunk 4 separate
                vt_sb = vtpool.tile([P, NCH, HID], F16)
                pv4 = psg.tile([P, 1024], F32, tag="g")
                for ch in range(4):
                    nc.tensor.matmul(
                        pv4[:].rearrange("p (c n) -> p c n", n=HID)[:, ch],
                        xt[:, ch * P:(ch + 1) * P], vwT[:],
                        start=True, stop=True)
                nc.vector.tensor_copy(
                    vt_sb[:, :4, :], pv4[:].rearrange("p (c n) -> p c n", n=HID))
                pv = psg.tile([P, 1024], F32, tag="g")
                nc.tensor.matmul(pv[:, :HID], xt[:, 4 * P:5 * P], vwT[:],
                                 start=True, stop=True)
                nc.vector.tensor_copy(vt_sb[:, 4, :], pv[:, :HID])
                vt_tiles[t] = vt_sb

                # scores + exp + bias-mult per (chunk, head)
                ex_sb = expool.tile([P, NCH, HEADS * L], BF16)
                for ch in range(NCH):
                    for h in range(HEADS):
                        # K=64 matmuls; the two heads run on disjoint PE
                        # row-groups into different PSUM banks (concurrent)
                        psc = psg.tile([P, 1024], F32, tag="g")
                        for lh in range(2):
                            nc.tensor.matmul(
                                _gap(psc[:])[:, lh],
                                k_sb[h * QKD:(h + 1) * QKD, ch * P:(ch + 1) * P],
                                q_sb[h * QKD:(h + 1) * QKD, lh * PIECE:(lh + 1) * PIECE],
                                start=True, stop=True)
                        _last_exp[0] = nc.scalar.activation(
                            _pieces(ex_sb[:, ch, h * L:(h + 1) * L]), _gap(psc[:]), EXP)
                        # engine-balance the bias multiply: DVE is 2x faster
                        # per element but DVE is the busier engine
                        eng = nc.gpsimd if (ch in (0, 2, 4)) else nc.vector
                        eng.tensor_tensor(
                            ex_sb[:, ch, h * L:(h + 1) * L],
                            ex_sb[:, ch, h * L:(h + 1) * L],
                            biasT[:, ch, h * L:(h + 1) * L], MULT)
                ex_tiles[t] = ex_sb

            def tail(t):
                """Dense (ungated) PE work for seq t: sums + attn@v, then the
                1/sums broadcast and the normalize. Emitted after front(t+1)
                so these matmuls fill the PE while seq t+1's scores trickle
                through the exp gate."""
                ex_sb = ex_tiles[t]
                vt_sb = vt_tiles[t]
                # softmax denominators first (the 1/sums DMA bounce then
                # overlaps the attn@v chains): two wide ones-matmul chains in
                # one psum tile, the 128-col remainder in a second, 32x32
                # transposes straight out of PSUM, sparse reciprocal,
                # transpose back, broadcast via DRAM bounce
                strans = smpool.tile([32, 4 * PIECE], F32, tag="strans")
                xb = xbpool.tile([P, 4 * PIECE], BF16)
                psA = psg.tile([P, 1024], F32, tag="g")
                for off in (0, 512):
                    for ch in range(NCH):
                        nc.tensor.matmul(
                            psA[0:32, off:off + 512], ones32[:],
                            ex_sb[:, ch, off:off + 512],
                            start=(ch == 0), stop=(ch == NCH - 1))
                for off in (0, 512):
                    nc.vector.transpose(strans[:, off:off + 512],
                                        psA[0:32, off:off + 512])
                psB = psg.tile([P, 1024], F32, tag="g")
                for ch in range(NCH):
                    nc.tensor.matmul(
                        psB[0:32, 0:128], ones32[:],
                        ex_sb[:, ch, 1024:1152],
                        start=(ch == 0), stop=(ch == NCH - 1))
                nc.vector.transpose(strans[:, 1024:1152], psB[0:32, 0:128])
                sparse = smpool.tile([32, 4 * PIECE], BF16, tag="sparse")
                nc.gpsimd.memset(sparse[:], 0.0)
                with nc.allow_low_precision(reason="softmax 1/sums in bf16 is within tolerance"):
                    nc.vector.reciprocal(
                        sparse[:].rearrange("p (b s) -> p b s", s=32)[:, :, 0],
                        strans[:].rearrange("p (b s) -> p b s", s=32)[:, :, 0])
                invrow = smpool.tile([32, 4 * PIECE], BF16, tag="invrow")
                nc.vector.transpose(invrow[:], sparse[:])
                inv_dr = drpool.tile([1, 4 * PIECE], BF16)
                nc.sync.dma_start(inv_dr[:], invrow[0:1, :])
                nc.sync.dma_start(xb[:], inv_dr[:].to_broadcast((P, 4 * PIECE)))

                # out = vT.T @ expT: dense accumulation chains, all inputs ready
                a_sb = apool.tile([P, 2, L], F16)
                for h in range(HEADS):
                    pm = psg.tile([P, 1024], F32, tag="g")
                    for ch in range(NCH):
                        for lh in range(2):  # reuse loaded vT weights across pieces
                            nc.tensor.matmul(
                                _gap(pm[:])[:, lh],
                                vt_sb[:, ch, h * VD:(h + 1) * VD],
                                ex_sb[:, ch, h * L + lh * PIECE: h * L + (lh + 1) * PIECE],
                                start=(ch == 0), stop=(ch == NCH - 1))
                    nc.vector.tensor_tensor(
                        _pieces(a_sb[:, h, :]), _gap(pm[:]),
                        _pieces(xb[:, h * 2 * PIECE:(h * 2 + 2) * PIECE]), MULT)
                a_tiles[t] = a_sb

            for t in range(seqs):
                front(t)
                if t >= 1:
                    tail(t - 1)
            tail(seqs - 1)

            # ---------------- Phase B: MLP ----------------
            for t in range(seqs):
                a_sb = a_tiles[t]
                g_sb = mpool.tile([P, 2, L], F16, tag="g")
                for mt in range(2):
                    py1 = psb.tile([P, 1024], F32, tag="b")
                    for lh in range(2):
                        for kt in range(2):
                            nc.tensor.matmul(
                                _gap(py1[:])[:, lh],
                                w1T[:, kt, mt * P:(mt + 1) * P],
                                a_sb[:, kt, lh * PIECE:(lh + 1) * PIECE],
                                start=(kt == 0), stop=(kt == 1))
                    _g = nc.scalar.activation(
                        _pieces(g_sb[:, mt, :]), _gap(py1[:]), GELU,
                        bias=b1c[:, mt:mt + 1], scale=1.0)
                    if not _dep_done[0] and _last_exp[0] is not None:
                        # keep ACT exp->gelu strictly phase-ordered: the two live in
                        # different ACT table sets, and a mid-phase switch costs ~2.7us
                        add_dep_helper(_last_exp[0].ins, _g.ins, sync=False,
                                       reason="ACT table set phase order")
                        _dep_done[0] = True
                y_sb = mpool.tile([OUT_CH, L], F16, tag="y")
                py2 = psb.tile([P, 1024], F32, tag="b")
                for lh in range(2):
                    for kt in range(2):
                        nc.tensor.matmul(
                            _gap(py2[:])[:, lh], w2T[:, kt, :],
                            g_sb[:, kt, lh * PIECE:(lh + 1) * PIECE],
                            start=(kt == 0), stop=(kt == 1))
                nc.vector.tensor_scalar_add(
                    _pieces(y_sb[:]), _gap(py2[:]), b2c[:, 0:1])
                nc.sync.dma_start(y_d[t], y_sb[:])
    nc.compile()
    return nc


_CACHED = {}


def _get_nc():
    if "nc" not in _CACHED:
        _CACHED["nc"] = build_kernel()
    return _CACHED["nc"]


def make_in_maps(x, aux):
    xr = np.ascontiguousarray(x.reshape(NSEQ, C, L)).astype(np.float16)
    in_maps = []
    for i in range(NCORES):
        m = {"x": xr[i * SEQ_PER_CORE:(i + 1) * SEQ_PER_CORE]}
        m.update(aux)
        in_maps.append(m)
    return in_maps


def kernel(x, qk_w, v_w, cpb_w1, cpb_b1, cpb_w2, sa_bias,
           mlp_w1, mlp_b1, mlp_w2, mlp_b2):
    x = np.asarray(x)
    aux = _prep_aux(np.asarray(qk_w), np.asarray(v_w), np.asarray(cpb_w1),
                    np.asarray(cpb_b1), np.asarray(cpb_w2), np.asarray(sa_bias),
                    np.asarray(mlp_w1), np.asarray(mlp_b1), np.asarray(mlp_w2),
                    np.asarray(mlp_b2))
    nc = _get_nc()
    in_maps = make_in_maps(x, aux)
    res = run_bass_kernel_spmd(nc, in_maps, core_ids=list(range(NCORES)))
    y = np.concatenate([res.results[i]["y"] for i in range(NCORES)], axis=0)
    return y.astype(np.float32).reshape(B, S, OUT_CH, HH, WW)


if __name__ == "__main__":
    import reference
    inputs = reference.setup_inputs()
    inputs = {k: np.asarray(v) for k, v in inputs.items()}
    out = kernel(**inputs)
    exp = np.asarray(reference.reference(**reference.setup_inputs()))
    err = np.abs(out - exp).max() / np.abs(exp).max()
    print("Relative error:", err)


# revision 12
# speedup vs baseline: 1.2936x; 1.0876x over previous
"""Trainium2 Bass kernel for nn_AttentionNeuralOperator (dense_transformer), v2.

Strategy (8 NeuronCores, data-parallel over the b*s=64 sequences, 8 per core):
  All matmuls in 16-bit (fp16 front-end / bf16 post-exp) with fp32 PSUM:
  fp16 has tf32-class mantissa so scores keep precision; E=exp(s)*exp(b) is
  bf16 for range. LDWEIGHTS runs 4x faster than fp32 (FWL), DMA bytes halve.
  Per sequence (c=128, L=576, 2 heads, d_qk=64, d_v=128):
    qk  = qk_wT.T @ x            -> q,k (d, L) fp16        [PE] -> DVE copies
    vT  = x.T @ v_wT             -> v (m, d) fp16          [PE]
    scoresT = k_h.T q_h          -> (m-part, l-free)       [PE, heads on
                                    disjoint row-groups, concurrent]
    expT = exp(scoresT) -> bf16  [ACT], multiplied by host-precomputed
      exp(bias) table (bf16), split DVE/GpSimd for engine balance.
    sums via ones-matmul, reciprocal via 32x32 transpose trick (bf16),
      broadcast via DRAM bounce.
    out_h = vT_h.T @ expT_h, normalized by 1/sums           [PE + DVE]
    1x1-conv MLP (fp16 weights) with exact GELU             [PE + ACT]
  y is fp16 on device; host upcasts to fp32.
"""
import sys
sys.path.insert(0, "/opt/trn_rl_repo")
import numpy as np
import ml_dtypes

import concourse.bass as bass
import concourse.tile as tile
from concourse.tile import add_dep_helper
from concourse import bacc, mybir
from concourse.bass_utils import run_bass_kernel_spmd

P = 128
HEADS = 2
B, S, C, HH, WW = 2, 32, 128, 24, 24
L = HH * WW            # 576
LP = 640               # m padded to 5*128
NSEQ = B * S           # 64
NCORES = 8
SEQ_PER_CORE = NSEQ // NCORES  # 8
HID = 256
OUT_CH = 128
QKD = C // HEADS       # 64
VD = HID // HEADS      # 128
NCH = LP // P          # 5 m-chunks
PIECE = 288            # l-piece
F32 = mybir.dt.float32
F16 = mybir.dt.float16
BF16 = mybir.dt.bfloat16
NPBF16 = ml_dtypes.bfloat16


def _log_cpb_np(h, w, w1, b1, w2):
    """Host fp32 mirror of the reference CPB MLP, on the 47x47 delta grid."""
    dy = np.arange(-(h - 1), h, dtype=np.float32)
    dx = np.arange(-(w - 1), w, dtype=np.float32)
    rel = np.stack(np.meshgrid(dy, dx, indexing="ij"), axis=-1)     # (2h-1, 2w-1, 2)
    denom = np.array([max(h - 1, 1), max(w - 1, 1)], dtype=np.float32)
    rel = rel / denom * np.float32(8.0)
    rel = np.sign(rel) * np.log2(np.float32(1.0) + np.abs(rel)) / np.float32(np.log2(8.0))
    hid_act = np.maximum(rel @ w1.T + b1, np.float32(0.0))          # (2h-1, 2w-1, c)
    tab = (hid_act @ w2.T).astype(np.float32)                       # (2h-1, 2w-1, heads)
    yl = np.repeat(np.arange(h), w)
    xl = np.tile(np.arange(w), h)
    DY = yl[:, None] - yl[None, :] + (h - 1)                        # (L, L)
    DX = xl[:, None] - xl[None, :] + (w - 1)
    return tab[DY, DX].transpose(2, 0, 1)                           # (heads, L, L)


def _prep_aux(qk_w, v_w, cpb_w1, cpb_b1, cpb_w2, sa_bias, mlp_w1, mlp_b1, mlp_w2, mlp_b2):
    scale = np.float32(1.0 / np.sqrt(QKD))
    qkwT = np.ascontiguousarray(qk_w.T).astype(np.float32).copy()   # (c, 2c)
    qkwT[:, :C] *= scale                                            # fold attn scale into q
    vwT = np.ascontiguousarray(v_w.T).astype(np.float32)            # (c, hid)

    bias = _log_cpb_np(HH, WW, cpb_w1, cpb_b1, cpb_w2)              # (heads, L, L)
    # multiplicative bias: exp(s+b) = exp(s)*exp(b); padded m-rows get 0 so
    # they vanish from the softmax sums and the attn@v contraction
    ebias = np.zeros((HEADS, LP, L), dtype=np.float32)
    ebias[:, :L, :] = np.exp(bias.transpose(0, 2, 1))               # [h, m, l]
    biasT_sb = np.empty((P, NCH, HEADS * L), dtype=np.float32)
    for ch in range(NCH):
        for h in range(HEADS):
            biasT_sb[:, ch, h * L:(h + 1) * L] = ebias[h, ch * P:(ch + 1) * P, :]

    w1T = np.empty((P, 2, HID), dtype=np.float32)                   # [p, kt, o]
    for kt in range(2):
        w1T[:, kt, :] = mlp_w1[:, kt * P:(kt + 1) * P].T
    w2T = np.empty((P, 2, OUT_CH), dtype=np.float32)
    for kt in range(2):
        w2T[:, kt, :] = mlp_w2[:, kt * P:(kt + 1) * P].T
    b1c = (mlp_w1 @ sa_bias.reshape(-1) + mlp_b1).astype(np.float32).reshape(2, P).T.copy()  # (128, 2)
    b2c = mlp_b2.astype(np.float32).reshape(P, 1).copy()
    return {
        "qkwT": qkwT.astype(np.float16),
        "vwT": vwT.astype(np.float16),
        "biasT": biasT_sb.astype(NPBF16),
        "ones32": np.ones((P, 32), dtype=NPBF16),
        "w1T": w1T.astype(np.float16),
        "w2T": w2T.astype(np.float16),
        "b1c": b1c, "b2c": b2c,
    }


def _gap(ap):
    """View a (128, 1024) psum tile as (128, 2, 288): pieces at [0:288], [512:800]."""
    return ap.rearrange("p (g c) -> p g c", c=512)[:, :, :PIECE]


def _pieces(ap576):
    """View a contiguous (128, 576) AP as (128, 2, 288)."""
    return ap576.rearrange("p (g c) -> p g c", c=PIECE)


def build_kernel(seqs=SEQ_PER_CORE, num_devices=NCORES):
    nc = bacc.Bacc("TRN2", target_bir_lowering=False, debug=False,
                   num_devices=num_devices)
    x_d = nc.dram_tensor("x", [seqs, C, L], F16, kind="ExternalInput").ap()
    qkwT_d = nc.dram_tensor("qkwT", [C, 2 * C], F16, kind="ExternalInput").ap()
    vwT_d = nc.dram_tensor("vwT", [C, HID], F16, kind="ExternalInput").ap()
    biasT_d = nc.dram_tensor("biasT", [P, NCH, HEADS * L], BF16, kind="ExternalInput").ap()
    ones32_d = nc.dram_tensor("ones32", [P, 32], BF16, kind="ExternalInput").ap()
    w1T_d = nc.dram_tensor("w1T", [P, 2, HID], F16, kind="ExternalInput").ap()
    w2T_d = nc.dram_tensor("w2T", [P, 2, OUT_CH], F16, kind="ExternalInput").ap()
    b1c_d = nc.dram_tensor("b1c", [P, 2], F32, kind="ExternalInput").ap()
    b2c_d = nc.dram_tensor("b2c", [P, 1], F32, kind="ExternalInput").ap()
    y_d = nc.dram_tensor("y", [seqs, OUT_CH, L], F16, kind="ExternalOutput").ap()

    EXP = mybir.ActivationFunctionType.Exp
    GELU = mybir.ActivationFunctionType.Gelu
    MULT = mybir.AluOpType.mult

    with tile.TileContext(nc) as tc:
        with (
            tc.tile_pool(name="const", bufs=1) as cpool,
            tc.tile_pool(name="xin", bufs=seqs) as xpool,
            tc.tile_pool(name="qk", bufs=4) as qkpool,
            tc.tile_pool(name="vt", bufs=4) as vtpool,
            tc.tile_pool(name="ex", bufs=3) as expool,
            tc.tile_pool(name="sm", bufs=3) as smpool,
            tc.tile_pool(name="xb", bufs=3) as xbpool,
            tc.tile_pool(name="drb", bufs=2, space="DRAM") as drpool,
            tc.tile_pool(name="acts", bufs=seqs) as apool,
            tc.tile_pool(name="mlp", bufs=3) as mpool,
            tc.tile_pool(name="psg", bufs=3, space="PSUM") as psg,
            tc.tile_pool(name="psb", bufs=1, space="PSUM") as psb,
        ):
            qkwT = cpool.tile([C, 2 * C], F16)
            vwT = cpool.tile([C, HID], F16)
            biasT = cpool.tile([P, NCH, HEADS * L], BF16)
            ones32 = cpool.tile([P, 32], BF16)
            w1T = cpool.tile([P, 2, HID], F16)
            w2T = cpool.tile([P, 2, OUT_CH], F16)
            b1c = cpool.tile([P, 2], F32)
            b2c = cpool.tile([P, 1], F32)
            nc.sync.dma_start(qkwT[:], qkwT_d[:])
            nc.sync.dma_start(vwT[:], vwT_d[:])
            # per-seq x tiles; bufs=seqs so all DMAs prefetch immediately and
            # the first matmul only waits for x[0]
            x_tiles = []
            for t in range(seqs):
                xt = xpool.tile([C, LP], F16)
                nc.sync.dma_start(xt[:, :L], x_d[t])
                nc.gpsimd.memset(xt[:, L:].bitcast(F32), 0.0)
                x_tiles.append(xt)
            for sb_t, dr in ((ones32, ones32_d), (biasT, biasT_d),
                             (w1T, w1T_d), (w2T, w2T_d),
                             (b1c, b1c_d), (b2c, b2c_d)):
                nc.sync.dma_start(sb_t[:], dr[:])

            _last_exp = [None]
            _dep_done = [False]
            a_tiles = {}
            ex_tiles = {}
            vt_tiles = {}

            # ---------------- Phase A: attention ----------------
            def front(t):
                """qk/v projections + scores/exp/bias for seq t."""
                xt = x_tiles[t]
                # qk projection: q rows (h0 d, h1 d), k rows (h0 d, h1 d)
                q_sb = qkpool.tile([P, L], F16, tag="q")
                k_sb = qkpool.tile([P, LP], F16, tag="k")
                for mt, dst in ((0, q_sb[:, :L]), (1, k_sb[:, :L])):
                    pqk = psg.tile([P, 1024], F32, tag="g")
                    for lh in range(2):
                        nc.tensor.matmul(
                            _gap(pqk[:])[:, lh],
                            qkwT[:, mt * P:(mt + 1) * P],
                            xt[:, lh * PIECE:(lh + 1) * PIECE],
                            start=True, stop=True)
                    nc.vector.tensor_copy(_pieces(dst), _gap(pqk[:]))
                nc.gpsimd.memset(k_sb[:, L:].bitcast(F32), 0.0)

                # vT: (m-part chunks, 256 = both heads' d); chunks 0-3 in one
                # psum tile + one batched copy, chunk 4 separate
                vt_sb = vtpool.tile([P, NCH, HID], F16)
                pv4 = psg.tile([P, 1024], F32, tag="g")
                for ch in range(4):
                    nc.tensor.matmul(
                        pv4[:].rearrange("p (c n) -> p c n", n=HID)[:, ch],
                        xt[:, ch * P:(ch + 1) * P], vwT[:],
                        start=True, stop=True)
                nc.vector.tensor_copy(
                    vt_sb[:, :4, :], pv4[:].rearrange("p (c n) -> p c n", n=HID))
                pv = psg.tile([P, 1024], F32, tag="g")
                nc.tensor.matmul(pv[:, :HID], xt[:, 4 * P:5 * P], vwT[:],
                                 start=True, stop=True)
                nc.vector.tensor_copy(vt_sb[:, 4, :], pv[:, :HID])
                vt_tiles[t] = vt_sb

                # scores + exp + bias-mult per (chunk, head)
                ex_sb = expool.tile([P, NCH, HEADS * L], BF16)
                for ch in range(NCH):
                    for h in range(HEADS):
                        # K=64 matmuls; the two heads run on disjoint PE
                        # row-groups into different PSUM banks (concurrent)
                        psc = psg.tile([P, 1024], F32, tag="g")
                        for lh in range(2):
                            nc.tensor.matmul(
                                _gap(psc[:])[:, lh],
                                k_sb[h * QKD:(h + 1) * QKD, ch * P:(ch + 1) * P],
                                q_sb[h * QKD:(h + 1) * QKD, lh * PIECE:(lh + 1) * PIECE],
                                start=True, stop=True)
                        _last_exp[0] = nc.scalar.activation(
                            _pieces(ex_sb[:, ch, h * L:(h + 1) * L]), _gap(psc[:]), EXP)
                        # engine-balance the bias multiply: DVE is 2x faster
                        # per element but DVE is the busier engine
                        eng = nc.gpsimd if (ch in (0, 2, 4)) else nc.vector
                        eng.tensor_tensor(
                            ex_sb[:, ch, h * L:(h + 1) * L],
                            ex_sb[:, ch, h * L:(h + 1) * L],
                            biasT[:, ch, h * L:(h + 1) * L], MULT)
                ex_tiles[t] = ex_sb

            def tail(t):
                """Dense (ungated) PE work for seq t: sums + attn@v, then the
                1/sums broadcast and the normalize. Emitted after front(t+1)
                so these matmuls fill the PE while seq t+1's scores trickle
                through the exp gate."""
                ex_sb = ex_tiles[t]
                vt_sb = vt_tiles[t]
                # softmax denominators first (the 1/sums DMA bounce then
                # overlaps the attn@v chains): two wide ones-matmul chains in
                # one psum tile, the 128-col remainder in a second, 32x32
                # transposes straight out of PSUM, sparse reciprocal,
                # transpose back, broadcast via DRAM bounce
                strans = smpool.tile([32, 4 * PIECE], F32, tag="strans")
                xb = xbpool.tile([P, 4 * PIECE], BF16)
                psA = psg.tile([P, 1024], F32, tag="g")
                for off in (0, 512):
                    for ch in range(NCH):
                        nc.tensor.matmul(
                            psA[0:32, off:off + 512], ones32[:],
                            ex_sb[:, ch, off:off + 512],
                            start=(ch == 0), stop=(ch == NCH - 1))
                for off in (0, 512):
                    nc.vector.transpose(strans[:, off:off + 512],
                                        psA[0:32, off:off + 512])
                psB = psg.tile([P, 1024], F32, tag="g")
                for ch in range(NCH):
                    nc.tensor.matmul(
                        psB[0:32, 0:128], ones32[:],
                        ex_sb[:, ch, 1024:1152],
                        start=(ch == 0), stop=(ch == NCH - 1))
                nc.vector.transpose(strans[:, 1024:1152], psB[0:32, 0:128])
                sparse = smpool.tile([32, 4 * PIECE], BF16, tag="sparse")
                nc.gpsimd.memset(sparse[:], 0.0)
                with nc.allow_low_precision(reason="softmax 1/sums in bf16 is within tolerance"):
                    nc.vector.reciprocal(
                        sparse[:].rearrange("p (b s) -> p b s", s=32)[:, :, 0],
                        strans[:].rearrange("p (b s) -> p b s", s=32)[:, :, 0])
                invrow = smpool.tile([32, 4 * PIECE], BF16, tag="invrow")
                nc.vector.transpose(invrow[:], sparse[:])
                inv_dr = drpool.tile([1, 4 * PIECE], BF16)
                nc.sync.dma_start(inv_dr[:], invrow[0:1, :])
                nc.sync.dma_start(xb[:], inv_dr[:].to_broadcast((P, 4 * PIECE)))

                # out = vT.T @ expT: dense accumulation chains, all inputs ready
                a_sb = apool.tile([P, 2, L], F16)
                for h in range(HEADS):
                    pm = psg.tile([P, 1024], F32, tag="g")
                    for ch in range(NCH):
                        for lh in range(2):  # reuse loaded vT weights across pieces
                            nc.tensor.matmul(
                                _gap(pm[:])[:, lh],
                                vt_sb[:, ch, h * VD:(h + 1) * VD],
                                ex_sb[:, ch, h * L + lh * PIECE: h * L + (lh + 1) * PIECE],
                                start=(ch == 0), stop=(ch == NCH - 1))
                    nc.vector.tensor_tensor(
                        _pieces(a_sb[:, h, :]), _gap(pm[:]),
                        _pieces(xb[:, h * 2 * PIECE:(h * 2 + 2) * PIECE]), MULT)
                a_tiles[t] = a_sb

            for t in range(seqs):
                front(t)
                if t >= 1:
                    tail(t - 1)
            tail(seqs - 1)

            # ---------------- Phase B: MLP ----------------
            for t in range(seqs):
                a_sb = a_tiles[t]
                g_sb = mpool.tile([P, 2, L], F16, tag="g")
                for mt in range(2):
                    py1 = psb.tile([P, 1024], F32, tag="b")
                    for lh in range(2):
                        for kt in range(2):
                            nc.tensor.matmul(
                                _gap(py1[:])[:, lh],
                                w1T[:, kt, mt * P:(mt + 1) * P],
                                a_sb[:, kt, lh * PIECE:(lh + 1) * PIECE],
                                start=(kt == 0), stop=(kt == 1))
                    _g = nc.scalar.activation(
                        _pieces(g_sb[:, mt, :]), _gap(py1[:]), GELU,
                        bias=b1c[:, mt:mt + 1], scale=1.0)
                    if not _dep_done[0] and _last_exp[0] is not None:
                        # keep ACT exp->gelu strictly phase-ordered: the two live in
                        # different ACT table sets, and a mid-phase switch costs ~2.7us
                        add_dep_helper(_last_exp[0].ins, _g.ins, sync=False,
                                       reason="ACT table set phase order")
                        _dep_done[0] = True
                y_sb = mpool.tile([OUT_CH, L], F16, tag="y")
                py2 = psb.tile([P, 1024], F32, tag="b")
                for lh in range(2):
                    for kt in range(2):
                        nc.tensor.matmul(
                            _gap(py2[:])[:, lh], w2T[:, kt, :],
                            g_sb[:, kt, lh * PIECE:(lh + 1) * PIECE],
                            start=(kt == 0), stop=(kt == 1))
                nc.vector.tensor_scalar_add(
                    _pieces(y_sb[:]), _gap(py2[:]), b2c[:, 0:1])
                nc.sync.dma_start(y_d[t], y_sb[:])
    nc.compile()
    return nc


_CACHED = {}


def _get_nc():
    if "nc" not in _CACHED:
        _CACHED["nc"] = build_kernel()
    return _CACHED["nc"]


def make_in_maps(x, aux):
    xr = np.ascontiguousarray(x.reshape(NSEQ, C, L)).astype(np.float16)
    in_maps = []
    for i in range(NCORES):
        m = {"x": xr[i * SEQ_PER_CORE:(i + 1) * SEQ_PER_CORE]}
        m.update(aux)
        in_maps.append(m)
    return in_maps


def kernel(x, qk_w, v_w, cpb_w1, cpb_b1, cpb_w2, sa_bias,
           mlp_w1, mlp_b1, mlp_w2, mlp_b2):
    x = np.asarray(x)
    aux = _prep_aux(np.asarray(qk_w), np.asarray(v_w), np.asarray(cpb_w1),
                    np.asarray(cpb_b1), np.asarray(cpb_w2), np.asarray(sa_bias),
                    np.asarray(mlp_w1), np.asarray(mlp_b1), np.asarray(mlp_w2),
                    np.asarray(mlp_b2))
    nc = _get_nc()
    in_maps = make_in_maps(x, aux)
    res = run_bass_kernel_spmd(nc, in_maps, core_ids=list(range(NCORES)))
    y = np.concatenate([res.results[i]["y"] for i in range(NCORES)], axis=0)
    return y.astype(np.float32).reshape(B, S, OUT_CH, HH, WW)


if __name__ == "__main__":
    import reference
    inputs = reference.setup_inputs()
    inputs = {k: np.asarray(v) for k, v in inputs.items()}
    out = kernel(**inputs)
    exp = np.asarray(reference.reference(**reference.setup_inputs()))
    err = np.abs(out - exp).max() / np.abs(exp).max()
    print("Relative error:", err)
